# revision 6
# baseline (speedup 1.0000x reference)
"""Trainium2 Bass kernel for nn_K24_RNNAgent_1R3 (moe_routing).

Strategy: MoE-style agent routing. Host sorts the 8192 rows (B*A) by
agent id and assigns agent `a`'s rows to core `a` (padded to a common
static capacity C). Each core computes ONLY its own agent's 2:4-softmax
masks + masked weights (sharding the mask softmax 8x) and runs the
fc1 -> GRU -> fc2 -> fc3 -> fc4 chain on its rows with dense fp32r
matmuls. Activations are kept feature-major [feature, rows] so matmul
contraction always runs over partitions and the GRU's gi+gh add happens
for free in PSUM. Host scatters the per-core outputs back.
"""

import numpy as np

from concourse import bacc, mybir, tile
import concourse.bass as bass
from concourse.bass_utils import run_bass_kernel_spmd
from concourse.masks import make_identity

F32 = mybir.dt.float32
F32R = mybir.dt.float32r
AF = mybir.ActivationFunctionType
OP = mybir.AluOpType

E, H, NA, A = 64, 512, 16, 8
TEMP = 5.0
INV_TEMP = 1.0 / TEMP


def _row_chunks(C):
    """Split C (multiple of 128) into <=512-col chunks, as equal as possible."""
    tiles = C // 128
    n = max(1, -(-C // 512))
    sizes = []
    for i in range(n):
        t = tiles // n + (1 if i < tiles % n else 0)
        if t:
            sizes.append(t * 128)
    return sizes


def _ap(a, extra_off, dims):
    return bass.AP(tensor=a.tensor, offset=a.offset + extra_off, ap=dims)


def _build_mask_wmT(nc, tc, pools, a_dram, wT_sbuf, wmT_sbuf, O, G, identity, add_eng):
    """Compute per-agent masked weight, transposed: wmT[i, o] = w[o, i] * mask[o, i].

    mask[o, g, k] = (sum_{p: k in p} exp(alpha[o,g,p]/T)) / (sum_p exp(...))
    with the 6 keep-2-of-4 patterns. alpha dram layout [O, G, 6].
    wT_sbuf: [ko, n_it, O] f32r (w transposed, i on partitions).
    wmT_sbuf: same shape f32r.
    """
    I = 4 * G
    n_ot = max(1, O // 128)
    P = min(O, 128)
    n_it = max(1, I // 128)
    pe, pm, psc, ptp = pools["e"], pools["mn"], pools["scal"], pools["tp"]
    for ot in range(n_ot):
        e = pe.tile([P, G, 6], F32, tag="e")
        nc.sync.dma_start(out=e, in_=a_dram.ap()[ot * 128:ot * 128 + P, :, :])
        # e = exp(alpha / TEMP)
        nc.scalar.activation(out=e, in_=e, func=AF.Exp, scale=INV_TEMP)
        mn = pm.tile([P, G, 4], F32, tag="mn")
        # numerators: k0 sums p{0,1,2}, k1 p{0,3,4}, k2 p{1,3,5}, k3 p{2,4,5}
        eb = e[:, :, :]
        p0 = eb.ap[0]
        out01 = mn[:, :, 0:2]
        add_eng.tensor_add(out=out01, in0=_ap(eb, 0, [p0, [6, G], [0, 2]]),
                           in1=_ap(eb, 1, [p0, [6, G], [2, 2]]))
        add_eng.tensor_add(out=out01, in0=out01, in1=_ap(eb, 2, [p0, [6, G], [2, 2]]))
        out23 = mn[:, :, 2:4]
        add_eng.tensor_add(out=out23, in0=_ap(eb, 1, [p0, [6, G], [1, 2]]),
                           in1=_ap(eb, 3, [p0, [6, G], [1, 2]]))
        add_eng.tensor_add(out=out23, in0=out23, in1=_ap(eb, 5, [p0, [6, G], [0, 2]]))
        # denominator: sum_k mn = 2 * sum_p e
        s2 = psc.tile([P, G], F32, tag="s2")
        nc.vector.tensor_reduce(out=s2, in_=mn, axis=mybir.AxisListType.X, op=OP.add)
        rcp = psc.tile([P, G], F32, tag="rcp")
        scr = psc.tile([P, G], F32, tag="scr")
        nc.vector.reciprocal_approx_accurate(out=rcp, in_=s2, scratch=scr)
        # mn = (mn * 2) * (1/s2)  (= mn / sum_p e)
        rb = rcp[:, :]
        nc.vector.scalar_tensor_tensor(
            out=mn, in0=mn, scalar=2.0, in1=_ap(rb, 0, [rb.ap[0], [1, G], [0, 4]]),
            op0=OP.mult, op1=OP.mult)
        # transpose 128-col blocks of mn (viewed [P, I]) and multiply by wT
        mn_ap = mn[:, :, :]
        mnf = _ap(mn_ap, 0, [mn_ap.ap[0], [1, I]])
        for it in range(n_it):
            cols = min(128, I - it * 128)
            ps = ptp.tile([cols, P], F32, tag="tp")
            nc.tensor.transpose(ps, mnf[:, it * 128:it * 128 + cols],
                                identity[0:P, 0:P])
            mt = pm.tile([cols, P], F32, tag="maskT")
            nc.scalar.copy(out=mt, in_=ps)
            nc.vector.tensor_mul(
                out=wmT_sbuf[0:cols, it, ot * 128:ot * 128 + P],
                in0=mt,
                in1=wT_sbuf[0:cols, it, ot * 128:ot * 128 + P].bitcast(F32))


def build_nc(C, n_iters=1):
    """Build the SPMD program for row-capacity C. If n_iters > 1, the whole
    body runs in an on-device loop (for wall-clock timing only)."""
    chunks = _row_chunks(C)
    nc = bacc.Bacc("TRN2", target_bir_lowering=False, debug=False, num_devices=8)

    dt = {}
    dram = {}
    for name, shape, d in [
        ("xT", [E, C], F32R), ("hT", [H, C], F32R),
        ("w1T", [E, H], F32R), ("wihT", [H, 3 * H], F32R), ("whhT", [H, 3 * H], F32R),
        ("w2T", [H, H], F32R), ("w3T", [H, H], F32R), ("w4T", [H, NA], F32R),
        ("a1", [H, E // 4, 6], F32), ("a2", [H, H // 4, 6], F32),
        ("a3", [H, H // 4, 6], F32), ("a4", [NA, H // 4, 6], F32),
        ("b1", [H], F32), ("bih", [3 * H], F32), ("bhh", [3 * H], F32),
        ("b2", [H], F32), ("b3", [H], F32), ("b4", [NA], F32),
    ]:
        dram[name] = nc.dram_tensor(name, shape, d, kind="ExternalInput")
    dram["hTo"] = nc.dram_tensor("hTo", [H, C], F32R, kind="ExternalOutput")
    dram["qTo"] = nc.dram_tensor("qTo", [NA, C], F32R, kind="ExternalOutput")

    with tile.TileContext(nc) as tc:
        import contextlib
        with contextlib.ExitStack() as ctx:
            p_const = ctx.enter_context(tc.tile_pool(name="const", bufs=1))
            p_w = ctx.enter_context(tc.tile_pool(name="weights", bufs=1))
            p_act = ctx.enter_context(tc.tile_pool(name="acts", bufs=1))
            p_e = ctx.enter_context(tc.tile_pool(name="mask_e", bufs=1))
            p_mn = ctx.enter_context(tc.tile_pool(name="mask_mn", bufs=1))
            p_scal = ctx.enter_context(tc.tile_pool(name="mask_scal", bufs=2))
            p_gates = ctx.enter_context(tc.tile_pool(name="gates", bufs=1))
            p_tmp = ctx.enter_context(tc.tile_pool(name="tmp", bufs=2))
            p_ps = ctx.enter_context(tc.tile_pool(name="ps", bufs=4, space="PSUM"))
            p_ps_n = ctx.enter_context(tc.tile_pool(name="ps_n", bufs=2, space="PSUM"))
            p_ps_tp = ctx.enter_context(tc.tile_pool(name="ps_tp", bufs=2, space="PSUM"))
            pools = {"e": p_e, "mn": p_mn, "scal": p_scal, "tp": p_ps_tp}

            identity = p_const.tile([128, 128], F32)
            make_identity(nc, identity)

            def body():
                # ---- load weights & biases ----
                w1T = p_w.tile([E, 1, H], F32R, tag="w1T")
                nc.sync.dma_start(out=w1T[:, 0, :], in_=dram["w1T"].ap())
                wihT = p_w.tile([128, 4, 3 * H], F32R, tag="wihT")
                whhT = p_w.tile([128, 4, 3 * H], F32R, tag="whhT")
                for kt in range(4):
                    nc.sync.dma_start(out=whhT[:, kt, :],
                                      in_=dram["whhT"].ap()[kt * 128:(kt + 1) * 128, :])
                    nc.sync.dma_start(out=wihT[:, kt, :],
                                      in_=dram["wihT"].ap()[kt * 128:(kt + 1) * 128, :])
                bcols = p_const.tile([128, 4 + 12 + 12 + 8 + 4 + 4], F32, tag="bcols")
                b1c, bihc, bhhc = bcols[:, 0:4], bcols[:, 4:16], bcols[:, 16:28]
                brz, b2c, b3c = bcols[:, 28:36], bcols[:, 36:40], bcols[:, 40:44]
                for (nm, c, n) in [("b1", b1c, 4), ("bih", bihc, 12), ("bhh", bhhc, 12),
                                   ("b2", b2c, 4), ("b3", b3c, 4)]:
                    nc.sync.dma_start(out=c, in_=_ap(dram[nm].ap(), 0, [[1, 128], [128, n]]))
                b4c = p_const.tile([NA, 1], F32, tag="b4c")
                nc.sync.dma_start(out=b4c, in_=_ap(dram["b4"].ap(), 0, [[1, NA], [1, 1]]))
                nc.vector.tensor_add(out=brz, in0=bihc[:, 0:8], in1=bhhc[:, 0:8])

                xT = p_act.tile([E, C], F32R, tag="xT")
                nc.sync.dma_start(out=xT, in_=dram["xT"].ap())
                hT = p_act.tile([128, 4, C], F32R, tag="hT")
                for kt in range(4):
                    nc.sync.dma_start(out=hT[:, kt, :],
                                      in_=dram["hT"].ap()[kt * 128:(kt + 1) * 128, :])

                # ---- fc1 mask + gemm -> x1T ----
                wm1T = p_w.tile([E, 1, H], F32R, tag="wm1T")
                _build_mask_wmT(nc, tc, pools, dram["a1"], w1T, wm1T, H, E // 4,
                                identity, nc.vector)
                x1T = p_act.tile([128, 4, C], F32R, tag="actA")
                c0 = 0
                for cs in chunks:
                    for ot in range(4):
                        ps = p_ps.tile([128, cs], F32, tag="mm")
                        nc.tensor.matmul(ps, wm1T[:, 0, ot * 128:(ot + 1) * 128],
                                         xT[:, c0:c0 + cs], start=True, stop=True)
                        nc.scalar.activation(out=x1T[:, ot, c0:c0 + cs], in_=ps,
                                             func=AF.Relu, bias=b1c[:, ot:ot + 1])
                    c0 += cs

                # ---- fc2/fc3/fc4 masked weights (overlaps GRU on DVE/ACT/PE) ----
                w2T = p_w.tile([128, 4, H], F32R, tag="wAB")
                for kt in range(4):
                    nc.sync.dma_start(out=w2T[:, kt, :],
                                      in_=dram["w2T"].ap()[kt * 128:(kt + 1) * 128, :])
                wm2T = p_w.tile([128, 4, H], F32R, tag="wm2T")
                _build_mask_wmT(nc, tc, pools, dram["a2"], w2T, wm2T, H, H // 4,
                                identity, nc.vector)

                # ---- GRU ----
                hpT = p_act.tile([128, 4, C], F32R, tag="hpT")
                c0 = 0
                for ci, cs in enumerate(chunks):
                    r_t = p_gates.tile([128, 4, cs], F32, tag="r")
                    z_t = p_gates.tile([128, 4, cs], F32, tag="z")
                    n_t = p_gates.tile([128, 4, cs], F32, tag="n")
                    for g, gt in ((0, r_t), (1, z_t)):
                        for oti in range(4):
                            ot = 4 * g + oti
                            ps = p_ps.tile([128, cs], F32, tag="mm")
                            for kt in range(4):
                                nc.tensor.matmul(ps, whhT[:, kt, ot * 128:(ot + 1) * 128],
                                                 hT[:, kt, c0:c0 + cs],
                                                 start=(kt == 0), stop=False)
                            for kt in range(4):
                                nc.tensor.matmul(ps, wihT[:, kt, ot * 128:(ot + 1) * 128],
                                                 x1T[:, kt, c0:c0 + cs],
                                                 start=False, stop=(kt == 3))
                            nc.scalar.activation(out=gt[:, oti, :], in_=ps,
                                                 func=AF.Sigmoid,
                                                 bias=brz[:, ot:ot + 1])
                    for oti in range(4):
                        ot = 8 + oti
                        ps_in = p_ps_n.tile([128, cs], F32, tag="nn")
                        ps_hn = p_ps_n.tile([128, cs], F32, tag="nn")
                        for kt in range(4):
                            nc.tensor.matmul(ps_hn, whhT[:, kt, ot * 128:(ot + 1) * 128],
                                             hT[:, kt, c0:c0 + cs],
                                             start=(kt == 0), stop=(kt == 3))
                        for kt in range(4):
                            nc.tensor.matmul(ps_in, wihT[:, kt, ot * 128:(ot + 1) * 128],
                                             x1T[:, kt, c0:c0 + cs],
                                             start=(kt == 0), stop=(kt == 3))
                        # t = (hn + bhh_n) * r ; narg = (in + bih_n) + t ; n = tanh(narg)
                        tt = p_tmp.tile([128, cs], F32, tag="tt")
                        nc.vector.scalar_tensor_tensor(
                            out=tt, in0=ps_hn, scalar=bhhc[:, ot:ot + 1],
                            in1=r_t[:, oti, :], op0=OP.add, op1=OP.mult)
                        nc.vector.scalar_tensor_tensor(
                            out=tt, in0=ps_in, scalar=bihc[:, ot:ot + 1],
                            in1=tt, op0=OP.add, op1=OP.add)
                        nc.scalar.activation(out=n_t[:, oti, :], in_=tt, func=AF.Tanh)
                    # h' = n + z * (h - n)
                    for oti in range(4):
                        d_t = p_tmp.tile([128, cs], F32, tag="d")
                        nc.gpsimd.tensor_sub(out=d_t,
                                             in0=hT[:, oti, c0:c0 + cs].bitcast(F32),
                                             in1=n_t[:, oti, :])
                        m_t = p_tmp.tile([128, cs], F32, tag="m")
                        nc.gpsimd.tensor_mul(out=m_t, in0=z_t[:, oti, :], in1=d_t)
                        nc.vector.tensor_add(out=hpT[:, oti, c0:c0 + cs],
                                             in0=n_t[:, oti, :], in1=m_t)
                    for kt in range(4):
                        nc.sync.dma_start(
                            out=dram["hTo"].ap()[kt * 128:(kt + 1) * 128, c0:c0 + cs],
                            in_=hpT[:, kt, c0:c0 + cs])
                    c0 += cs

                # ---- fc3/fc4 masks ----
                w3T = p_w.tile([128, 4, H], F32R, tag="wAB")
                for kt in range(4):
                    nc.sync.dma_start(out=w3T[:, kt, :],
                                      in_=dram["w3T"].ap()[kt * 128:(kt + 1) * 128, :])
                wm3T = p_w.tile([128, 4, H], F32R, tag="wm3T")
                _build_mask_wmT(nc, tc, pools, dram["a3"], w3T, wm3T, H, H // 4,
                                identity, nc.gpsimd)
                w4T = p_w.tile([128, 4, NA], F32R, tag="w4T")
                nc.sync.dma_start(out=w4T,
                                  in_=_ap(dram["w4T"].ap(), 0, [[NA, 128], [NA * 128, 4], [1, NA]]))
                wm4T = p_w.tile([128, 4, NA], F32R, tag="wm4T")
                _build_mask_wmT(nc, tc, pools, dram["a4"], w4T, wm4T, NA, H // 4,
                                identity, nc.vector)

                # ---- fc2 -> q2T, fc3 -> q3T, fc4 -> qTo ----
                q2T = p_act.tile([128, 4, C], F32R, tag="hT")
                c0 = 0
                for cs in chunks:
                    for ot in range(4):
                        ps = p_ps.tile([128, cs], F32, tag="mm")
                        for kt in range(4):
                            nc.tensor.matmul(ps, wm2T[:, kt, ot * 128:(ot + 1) * 128],
                                             hpT[:, kt, c0:c0 + cs],
                                             start=(kt == 0), stop=(kt == 3))
                        nc.scalar.activation(out=q2T[:, ot, c0:c0 + cs], in_=ps,
                                             func=AF.Relu, bias=b2c[:, ot:ot + 1])
                    c0 += cs
                q3T = p_act.tile([128, 4, C], F32R, tag="actA")
                c0 = 0
                for cs in chunks:
                    for ot in range(4):
                        ps = p_ps.tile([128, cs], F32, tag="mm")
                        for kt in range(4):
                            nc.tensor.matmul(ps, wm3T[:, kt, ot * 128:(ot + 1) * 128],
                                             q2T[:, kt, c0:c0 + cs],
                                             start=(kt == 0), stop=(kt == 3))
                        nc.scalar.activation(out=q3T[:, ot, c0:c0 + cs], in_=ps,
                                             func=AF.Relu, bias=b3c[:, ot:ot + 1])
                    c0 += cs
                q4 = p_act.tile([NA, C], F32R, tag="q4")
                c0 = 0
                for cs in chunks:
                    ps = p_ps.tile([NA, cs], F32, tag="mm")
                    for kt in range(4):
                        nc.tensor.matmul(ps, wm4T[:, kt, :], q3T[:, kt, c0:c0 + cs],
                                         start=(kt == 0), stop=(kt == 3))
                    nc.scalar.activation(out=q4[:, c0:c0 + cs], in_=ps,
                                         func=AF.Identity, bias=b4c[:, 0:1])
                    c0 += cs
                nc.sync.dma_start(out=dram["qTo"].ap(), in_=q4)

            if n_iters > 1:
                with tc.For_i(0, n_iters, 1):
                    body()
            else:
                body()

    nc.compile()
    return nc


_NC_CACHE = {}


def _get_nc(C, n_iters=1):
    key = (C, n_iters)
    if key not in _NC_CACHE:
        _NC_CACHE[key] = build_nc(C, n_iters)
    return _NC_CACHE[key]


def make_in_maps(inputs, hidden_state, agent_ids,
                 fc1_w, fc1_b, fc1_alpha, gru_w_ih, gru_w_hh, gru_b_ih, gru_b_hh,
                 fc2_w, fc2_b, fc2_alpha, fc3_w, fc3_b, fc3_alpha,
                 fc4_w, fc4_b, fc4_alpha):
    B, A_, E_ = inputs.shape
    N = B * A_
    ids = np.asarray(agent_ids).reshape(-1)
    xr = np.asarray(inputs, np.float32).reshape(N, E_)
    hr = np.asarray(hidden_state, np.float32).reshape(N, H)
    idxs = [np.nonzero(ids == a)[0] for a in range(A)]
    C = max(128, -(-max(len(i) for i in idxs) // 128) * 128)

    f32c = lambda a: np.ascontiguousarray(np.asarray(a, np.float32))
    shared = {
        "w1T": f32c(fc1_w.T), "wihT": f32c(gru_w_ih.T), "whhT": f32c(gru_w_hh.T),
        "w2T": f32c(fc2_w.T), "w3T": f32c(fc3_w.T), "w4T": f32c(fc4_w.T),
        "b1": f32c(fc1_b), "bih": f32c(gru_b_ih), "bhh": f32c(gru_b_hh),
        "b2": f32c(fc2_b), "b3": f32c(fc3_b), "b4": f32c(fc4_b),
    }
    a1, a2 = np.asarray(fc1_alpha, np.float32), np.asarray(fc2_alpha, np.float32)
    a3, a4 = np.asarray(fc3_alpha, np.float32), np.asarray(fc4_alpha, np.float32)
    in_maps = []
    for a in range(A):
        idx = idxs[a]
        xp = np.zeros((C, E_), np.float32)
        hp = np.zeros((C, H), np.float32)
        xp[:len(idx)] = xr[idx]
        hp[:len(idx)] = hr[idx]
        m = dict(shared)
        m["xT"] = np.ascontiguousarray(xp.T)
        m["hT"] = np.ascontiguousarray(hp.T)
        m["a1"], m["a2"] = f32c(a1[a]), f32c(a2[a])
        m["a3"], m["a4"] = f32c(a3[a]), f32c(a4[a])
        in_maps.append(m)
    return in_maps, idxs, C, N


def kernel(**inputs):
    in_maps, idxs, C, N = make_in_maps(**inputs)
    nc = _get_nc(C)
    res = run_bass_kernel_spmd(nc, in_maps, list(range(A)))
    q = np.empty((N, NA), np.float32)
    h = np.empty((N, H), np.float32)
    for a in range(A):
        idx = idxs[a]
        if len(idx):
            q[idx] = res.results[a]["qTo"][:, :len(idx)].T
            h[idx] = res.results[a]["hTo"][:, :len(idx)].T
    B, A_, _ = inputs["inputs"].shape
    return q.reshape(B, A_, NA), h.reshape(B, A_, H)
